# revision 16
# baseline (speedup 1.0000x reference)
"""Trainium2 Bass kernel for nn_DeformConv_23278722744918.

The reference passes raw integer pixel coordinates to grid_sample as if they
were normalized [-1,1] coords (align_corners=True). After de-normalization,
xpix = (clip(h+i,0,95)+1)*47.5 and ypix = (clip(w+j,0,95)+1)*47.5, so every
sample with h+i >= 2 or w+j >= 2 lands outside [0,95] and is zero
(padding_mode='zeros').  Only four tap values survive, shared by all (h,w):

  A = 0.25*(x[47,47]+x[47,48]+x[48,47]+x[48,48])   (coord cases 0,0)
  B = 0.50*(x[47,95]+x[48,95])                     (coord cases 1,0)
  C = 0.50*(x[95,47]+x[95,48])                     (coord cases 0,1)
  D =       x[95,95]                               (coord cases 1,1)

After the stride-3 VALID conv over the rearranged feature map, the output is
b_conv everywhere except the 2x2 corner (per batch, out-channel):

  out[b,o,0,0] = sum_c A*w00 + C*w01 + B*w10 + D*w11   (+ b_conv[o])
  out[b,o,0,1] = sum_c C*w00 + D*w10
  out[b,o,1,0] = sum_c B*w00 + D*w01
  out[b,o,1,1] = sum_c D*w00

(w_ij = w_conv[o,c,i,j]; the offset-conv branch is dead: + 0.0*sum(off).)

Sharding: output channels are split 8 ways across the NeuronCores (the batch
dim is only 4); the sampled rows of x (y=47,48,95) are replicated.  Each core
gathers its bilinear taps on-device (reduces + scales), runs the 4 corner
matmuls on the TensorEngine, fills its background tile and writes its
[4,8,96,96] output shard with disjoint DMAs spread over both HWDGE rings.

Two program variants: b_conv==0 (always true for this problem's
setup_inputs) uses a pure-memset background; nonzero b_conv broadcasts the
bias with a DVE copy (step-0 AP) instead.
"""

import numpy as np

B, IC, IH, IW = 4, 64, 96, 96
OC = 64
NCORES = 8
OCP = OC // NCORES  # out channels per core
HW = IH * IW        # 9216
QCH = HW // 4       # 2304: background tile free size (4 partition-chunks/plane)

_ROWS = (47, 48, 95)  # sampled rows of x (y coords); cols sampled: 47,48,95

_prog_cache = {}


def _build_program(with_bias):
    """One SPMD Bass program: identical on every core; per-core data differs."""
    import concourse.bacc as bacc
    import concourse.bass as bass
    import concourse.mybir as mybir
    import concourse.tile as tile

    nc = bacc.Bacc()
    dt = mybir.dt.float32

    xr_d = nc.declare_dram_parameter("xr", [IC, B, 3, IW], dt, isOutput=False)
    wb_d = nc.declare_dram_parameter("wb", [128, 64], dt, isOutput=False)
    if with_bias:
        b128_d = nc.declare_dram_parameter("bias128", [128, 1], dt, isOutput=False)
        bv_d = nc.declare_dram_parameter("biasV", [B, OCP], dt, isOutput=False)
    out_d = nc.declare_dram_parameter("out", [B, OCP, IH, IW], dt, isOutput=True)

    with tile.TileContext(nc) as tc:
        with (
            tc.tile_pool(name="sbuf", bufs=1) as pool,
            tc.tile_pool(name="psum", bufs=1, space=bass.MemorySpace.PSUM) as psum,
        ):
            xr = pool.tile([IC, B, 3, IW], dt)
            w2 = pool.tile([128, 64], dt)
            bgh = pool.tile([B, OCP, 192], dt)
            S3 = pool.tile([128, 4], dt)
            S4 = pool.tile([128, 4], dt)
            bg = pool.tile([128, QCH], dt)
            Vp = psum.tile([B, 32], dt)

            # Background: bg[q*32 + b*8 + o, r] = b_conv[o] (or just zeros).
            # The fill gates the big output writes, so keep it off the Scalar
            # engine (ACT table load + drain costs ~6us) and split the plain
            # memset across DVE and GpSimd.
            nc.scalar.dma_start(xr[:], xr_d[:])
            nc.scalar.dma_start(w2[:], wb_d[:])

            if with_bias:
                b128 = pool.tile([128, 1], dt)
                bo8 = pool.tile([B, OCP], dt)
                nc.scalar.dma_start(b128[:], b128_d[:])
                nc.scalar.dma_start(bo8[:], bv_d[:])
                nc.vector.tensor_copy(bg[:], b128[:, 0:1].to_broadcast((128, QCH)))
                nc.vector.tensor_copy(
                    bgh[:], bo8[:, :, None].to_broadcast((B, OCP, 192))
                )
            else:
                nc.vector.memset(bg[:, 0 : QCH // 2], 0.0)
                nc.gpsimd.memset(bg[:, QCH // 2 : QCH], 0.0)
                nc.gpsimd.memset(bgh[:], 0.0)

            # Bilinear tap sums (bilinear scale factors are folded into the
            # host-prepared weight matrix).  Taps stacked on the K axis:
            # S3 = [A (c rows 0:64) | B (rows 64:128)], S4 = [C | D],
            # free dim = b.  xr rows: 0->y47, 1->y48, 2->y95.
            AX = mybir.AxisListType
            nc.vector.reduce_sum(S3[0:64, :], xr[:, :, 0:2, 47:49], axis=AX.XY)
            nc.vector.reduce_sum(S3[64:128, :], xr[:, :, 0:2, 95:96], axis=AX.XY)
            nc.vector.reduce_sum(S4[0:64, :], xr[:, :, 2:3, 47:49], axis=AX.XY)
            nc.vector.tensor_copy(S4[64:128, :], xr[:, :, 2:3, 95:96])

            # Corner values Vp[b, (h*2+w)*8+o] in two K=128 matmuls; w2 holds
            # the scale-folded, zero-padded tap weights (cols 0:32 for A|B,
            # 32:64 for C|D).
            MM = nc.tensor.matmul
            MM(Vp[:], S3[:], w2[:, 0:32], start=True, stop=False)
            MM(Vp[:], S4[:], w2[:, 32:64], start=False, stop=True)

            # Add the corner values straight from PSUM into the q0-head
            # tile (rows h=0,1 of every (b,o) plane): bgh[b, o, h*96+w] +=
            # Vp[b, (h*2+w)*8+o].  One DVE op; no small-run DMA ever targets
            # HBM (their completion fences were the kernel's tail before).
            bghv = bgh[:].rearrange("b o (h x w) -> b o h x w", h=2, x=48)[
                :, :, :, 0, 0:2
            ]
            nc.vector.tensor_add(
                bghv,
                bghv,
                Vp[:].rearrange("b (h w o) -> b o h w", h=2, w=2),
            )

            # Output shard writes, all disjoint: plane hw = q*2304 + r.
            # Alternate the two HWDGE rings (sync=SP, scalar=ACT).
            ovb = out_d[:].rearrange("b o h w -> b o (h w)")
            ov = out_d[:].rearrange("b o h w -> (b o) (h w)")
            nc.sync.dma_start(ovb[:, :, 0:192], bgh[:])
            nc.scalar.dma_start(ov[:, 192:QCH], bg[0:32, 192:QCH])
            nc.sync.dma_start(ov[:, QCH : 2 * QCH], bg[32:64, :])
            nc.scalar.dma_start(ov[:, 2 * QCH : 3 * QCH], bg[64:96, :])
            nc.sync.dma_start(ov[:, 3 * QCH : 4 * QCH], bg[96:128, :])

    nc.finalize()  # Bacc.finalize runs the wait-splitting legalization passes
    return nc


def _get_program(with_bias):
    key = bool(with_bias)
    if key not in _prog_cache:
        _prog_cache[key] = _build_program(key)
    return _prog_cache[key]


def _make_in_maps(x, w_conv, b_conv, with_bias=None):
    x = np.ascontiguousarray(x, dtype=np.float32)
    w_conv = np.ascontiguousarray(w_conv, dtype=np.float32)
    b_conv = np.ascontiguousarray(b_conv, dtype=np.float32)
    if with_bias is None:
        with_bias = bool(np.any(b_conv != 0))

    xr = np.ascontiguousarray(x[:, :, _ROWS, :].transpose(1, 0, 2, 3))
    in_maps = []
    for core in range(NCORES):
        o0 = core * OCP
        wsl = w_conv[o0 : o0 + OCP, :, 0:2, 0:2]  # [8,64,2,2] (o,c,i,j)
        # w_pq[pq][c, o] helper: w_conv[o,c,i,j] transposed to [c,o]
        wco = lambda i, j: wsl[:, :, i, j].T  # [64(c), 8(o)]
        z = np.zeros((IC, OCP), np.float32)
        # Vp[b, pq*8+o] = sum_k S[k,b]*w2[k, pq*8+o]; K rows: A=0:64, B=64:128
        # (first matmul), C=0:64, D=64:128 (second).  Scales folded here.
        wA = np.concatenate([0.25 * wco(0, 0), z, z, z], axis=1)          # out00
        wB = np.concatenate([0.5 * wco(1, 0), z, 0.5 * wco(0, 0), z], axis=1)
        wC = np.concatenate([0.5 * wco(0, 1), 0.5 * wco(0, 0), z, z], axis=1)
        wD = np.concatenate([wco(1, 1), wco(1, 0), wco(0, 1), wco(0, 0)], axis=1)
        w2 = np.ascontiguousarray(
            np.concatenate(
                [np.concatenate([wA, wB], axis=0), np.concatenate([wC, wD], axis=0)],
                axis=1,
            )
        )  # [128, 64]
        m = {"xr": xr, "wb": w2}
        if with_bias:
            bc8 = b_conv[o0 : o0 + OCP]
            m["bias128"] = np.ascontiguousarray(np.tile(bc8, 16)[:, None])
            # biasV[b, o] = bc8[o]
            m["biasV"] = np.ascontiguousarray(
                np.broadcast_to(bc8[None, :], (B, OCP))
            )
        in_maps.append(m)
    return in_maps


def _run(x, w_conv, b_conv, trace=False, **spmd_kwargs):
    from concourse.bass_utils import run_bass_kernel_spmd

    with_bias = bool(np.any(np.asarray(b_conv) != 0))
    nc = _get_program(with_bias)
    in_maps = _make_in_maps(x, w_conv, b_conv, with_bias)
    res = run_bass_kernel_spmd(
        nc, in_maps, core_ids=list(range(NCORES)), trace=trace, **spmd_kwargs
    )
    out = np.concatenate([r["out"] for r in res.results], axis=1)
    return out, res


def kernel(x, w_off, b_off, w_conv, b_conv):
    out, _ = _run(x, w_conv, b_conv, trace=False)
    return out


# revision 17
# speedup vs baseline: 1.0470x; 1.0470x over previous
"""Trainium2 Bass kernel for nn_DeformConv_23278722744918.

The reference passes raw integer pixel coordinates to grid_sample as if they
were normalized [-1,1] coords (align_corners=True). After de-normalization,
xpix = (clip(h+i,0,95)+1)*47.5 and ypix = (clip(w+j,0,95)+1)*47.5, so every
sample with h+i >= 2 or w+j >= 2 lands outside [0,95] and is zero
(padding_mode='zeros').  Only four tap values survive, shared by all (h,w):

  A = 0.25*(x[47,47]+x[47,48]+x[48,47]+x[48,48])   (coord cases 0,0)
  B = 0.50*(x[47,95]+x[48,95])                     (coord cases 1,0)
  C = 0.50*(x[95,47]+x[95,48])                     (coord cases 0,1)
  D =       x[95,95]                               (coord cases 1,1)

After the stride-3 VALID conv over the rearranged feature map, the output is
b_conv everywhere except the 2x2 corner (per batch, out-channel):

  out[b,o,0,0] = sum_c A*w00 + C*w01 + B*w10 + D*w11   (+ b_conv[o])
  out[b,o,0,1] = sum_c C*w00 + D*w10
  out[b,o,1,0] = sum_c B*w00 + D*w01
  out[b,o,1,1] = sum_c D*w00

(w_ij = w_conv[o,c,i,j]; the offset-conv branch is dead: + 0.0*sum(off).)

Sharding: output channels are split 8 ways across the NeuronCores (the batch
dim is only 4); the sampled rows of x (y=47,48,95) are replicated.  Each core
gathers its bilinear taps on-device (reduces + scales), runs the 4 corner
matmuls on the TensorEngine, fills its background tile and writes its
[4,8,96,96] output shard with disjoint DMAs spread over both HWDGE rings.

Two program variants: b_conv==0 (always true for this problem's
setup_inputs) uses a pure-memset background; nonzero b_conv broadcasts the
bias with a DVE copy (step-0 AP) instead.
"""

import numpy as np

B, IC, IH, IW = 4, 64, 96, 96
OC = 64
NCORES = 8
OCP = OC // NCORES  # out channels per core
HW = IH * IW        # 9216
QCH = HW // 4       # 2304: background tile free size (4 partition-chunks/plane)

_ROWS = (47, 48, 95)  # sampled rows of x (y coords); cols sampled: 47,48,95

_prog_cache = {}


def _build_program(with_bias):
    """One SPMD Bass program: identical on every core; per-core data differs."""
    import concourse.bacc as bacc
    import concourse.bass as bass
    import concourse.mybir as mybir
    import concourse.tile as tile

    nc = bacc.Bacc()
    dt = mybir.dt.float32

    xr_d = nc.declare_dram_parameter("xr", [IC, B, 3, 3], dt, isOutput=False)
    wb_d = nc.declare_dram_parameter("wb", [128, 64], dt, isOutput=False)
    if with_bias:
        b128_d = nc.declare_dram_parameter("bias128", [128, 1], dt, isOutput=False)
        bv_d = nc.declare_dram_parameter("biasV", [B, OCP], dt, isOutput=False)
    out_d = nc.declare_dram_parameter("out", [B, OCP, IH, IW], dt, isOutput=True)

    with tile.TileContext(nc) as tc:
        with (
            tc.tile_pool(name="sbuf", bufs=1) as pool,
            tc.tile_pool(name="psum", bufs=1, space=bass.MemorySpace.PSUM) as psum,
        ):
            xr = pool.tile([IC, B, 3, 3], dt)
            w2 = pool.tile([128, 64], dt)
            bgh = pool.tile([B, OCP, 192], dt)
            S3 = pool.tile([128, 4], dt)
            S4 = pool.tile([128, 4], dt)
            bg = pool.tile([128, QCH], dt)
            Vp = psum.tile([B, 32], dt)

            # Background: bg[q*32 + b*8 + o, r] = b_conv[o] (or just zeros).
            # The fill gates the big output writes, so keep it off the Scalar
            # engine (ACT table load + drain costs ~6us) and split the plain
            # memset across DVE and GpSimd.
            # Input DMAs first on both rings: their ~2-3us HBM completion
            # fences gate the whole corner-value chain.
            nc.scalar.dma_start(xr[:], xr_d[:])
            nc.sync.dma_start(w2[:], wb_d[:])

            if with_bias:
                b128 = pool.tile([128, 1], dt)
                bo8 = pool.tile([B, OCP], dt)
                nc.scalar.dma_start(b128[:], b128_d[:])
                nc.scalar.dma_start(bo8[:], bv_d[:])
                nc.vector.tensor_copy(bg[:], b128[:, 0:1].to_broadcast((128, QCH)))
                nc.vector.tensor_copy(
                    bgh[:], bo8[:, :, None].to_broadcast((B, OCP, 192))
                )
            else:
                nc.vector.memset(bg[:, 0 : QCH // 2], 0.0)
                nc.gpsimd.memset(bg[:, QCH // 2 : QCH], 0.0)
                nc.gpsimd.memset(bgh[:], 0.0)

            # Bilinear tap sums (bilinear scale factors are folded into the
            # host-prepared weight matrix).  Taps stacked on the K axis:
            # S3 = [A (c rows 0:64) | B (rows 64:128)], S4 = [C | D],
            # free dim = b.  xr rows: 0->y47, 1->y48, 2->y95.
            AX = mybir.AxisListType
            nc.vector.reduce_sum(S3[0:64, :], xr[:, :, 0:2, 0:2], axis=AX.XY)
            nc.vector.reduce_sum(S3[64:128, :], xr[:, :, 0:2, 2:3], axis=AX.XY)
            nc.vector.reduce_sum(S4[0:64, :], xr[:, :, 2:3, 0:2], axis=AX.XY)
            nc.vector.tensor_copy(S4[64:128, :], xr[:, :, 2:3, 2:3])

            # Corner values Vp[b, (h*2+w)*8+o] in two K=128 matmuls; w2 holds
            # the scale-folded, zero-padded tap weights (cols 0:32 for A|B,
            # 32:64 for C|D).
            MM = nc.tensor.matmul
            MM(Vp[:], S3[:], w2[:, 0:32], start=True, stop=False)
            MM(Vp[:], S4[:], w2[:, 32:64], start=False, stop=True)

            # Add the corner values straight from PSUM into the q0-head
            # tile (rows h=0,1 of every (b,o) plane): bgh[b, o, h*96+w] +=
            # Vp[b, (h*2+w)*8+o].  One DVE op; no small-run DMA ever targets
            # HBM (their completion fences were the kernel's tail before).
            bghv = bgh[:].rearrange("b o (h x w) -> b o h x w", h=2, x=48)[
                :, :, :, 0, 0:2
            ]
            nc.vector.tensor_add(
                bghv,
                bghv,
                Vp[:].rearrange("b (h w o) -> b o h w", h=2, w=2),
            )

            # Output shard writes, all disjoint: plane hw = q*2304 + r.
            # Alternate the two HWDGE rings (sync=SP, scalar=ACT).
            ovb = out_d[:].rearrange("b o h w -> b o (h w)")
            ov = out_d[:].rearrange("b o h w -> (b o) (h w)")
            nc.scalar.dma_start(ov[:, 192:QCH], bg[0:32, 192:QCH])
            nc.sync.dma_start(ov[:, QCH : 2 * QCH], bg[32:64, :])
            nc.scalar.dma_start(ov[:, 2 * QCH : 3 * QCH], bg[64:96, :])
            nc.sync.dma_start(ov[:, 3 * QCH : 4 * QCH], bg[96:128, :])
            # V-dependent head write last so it can't head-of-line block the
            # independent plane writes on its ring.
            nc.sync.dma_start(ovb[:, :, 0:192], bgh[:])

    nc.finalize()  # Bacc.finalize runs the wait-splitting legalization passes
    return nc


def _get_program(with_bias):
    key = bool(with_bias)
    if key not in _prog_cache:
        _prog_cache[key] = _build_program(key)
    return _prog_cache[key]


def _make_in_maps(x, w_conv, b_conv, with_bias=None):
    x = np.ascontiguousarray(x, dtype=np.float32)
    w_conv = np.ascontiguousarray(w_conv, dtype=np.float32)
    b_conv = np.ascontiguousarray(b_conv, dtype=np.float32)
    if with_bias is None:
        with_bias = bool(np.any(b_conv != 0))

    xr = np.ascontiguousarray(
        x[:, :, _ROWS, :][:, :, :, _ROWS].transpose(1, 0, 2, 3)
    )
    in_maps = []
    for core in range(NCORES):
        o0 = core * OCP
        wsl = w_conv[o0 : o0 + OCP, :, 0:2, 0:2]  # [8,64,2,2] (o,c,i,j)
        # w_pq[pq][c, o] helper: w_conv[o,c,i,j] transposed to [c,o]
        wco = lambda i, j: wsl[:, :, i, j].T  # [64(c), 8(o)]
        z = np.zeros((IC, OCP), np.float32)
        # Vp[b, pq*8+o] = sum_k S[k,b]*w2[k, pq*8+o]; K rows: A=0:64, B=64:128
        # (first matmul), C=0:64, D=64:128 (second).  Scales folded here.
        wA = np.concatenate([0.25 * wco(0, 0), z, z, z], axis=1)          # out00
        wB = np.concatenate([0.5 * wco(1, 0), z, 0.5 * wco(0, 0), z], axis=1)
        wC = np.concatenate([0.5 * wco(0, 1), 0.5 * wco(0, 0), z, z], axis=1)
        wD = np.concatenate([wco(1, 1), wco(1, 0), wco(0, 1), wco(0, 0)], axis=1)
        w2 = np.ascontiguousarray(
            np.concatenate(
                [np.concatenate([wA, wB], axis=0), np.concatenate([wC, wD], axis=0)],
                axis=1,
            )
        )  # [128, 64]
        m = {"xr": xr, "wb": w2}
        if with_bias:
            bc8 = b_conv[o0 : o0 + OCP]
            m["bias128"] = np.ascontiguousarray(np.tile(bc8, 16)[:, None])
            # biasV[b, o] = bc8[o]
            m["biasV"] = np.ascontiguousarray(
                np.broadcast_to(bc8[None, :], (B, OCP))
            )
        in_maps.append(m)
    return in_maps


def _run(x, w_conv, b_conv, trace=False, **spmd_kwargs):
    from concourse.bass_utils import run_bass_kernel_spmd

    with_bias = bool(np.any(np.asarray(b_conv) != 0))
    nc = _get_program(with_bias)
    in_maps = _make_in_maps(x, w_conv, b_conv, with_bias)
    res = run_bass_kernel_spmd(
        nc, in_maps, core_ids=list(range(NCORES)), trace=trace, **spmd_kwargs
    )
    out = np.concatenate([r["out"] for r in res.results], axis=1)
    return out, res


def kernel(x, w_off, b_off, w_conv, b_conv):
    out, _ = _run(x, w_conv, b_conv, trace=False)
    return out


# revision 18
# speedup vs baseline: 1.2395x; 1.1839x over previous
"""Trainium2 Bass kernel for nn_DeformConv_23278722744918.

The reference passes raw integer pixel coordinates to grid_sample as if they
were normalized [-1,1] coords (align_corners=True). After de-normalization,
xpix = (clip(h+i,0,95)+1)*47.5 and ypix = (clip(w+j,0,95)+1)*47.5, so every
sample with h+i >= 2 or w+j >= 2 lands outside [0,95] and is zero
(padding_mode='zeros').  Only four tap values survive, shared by all (h,w):

  A = 0.25*(x[47,47]+x[47,48]+x[48,47]+x[48,48])   (coord cases 0,0)
  B = 0.50*(x[47,95]+x[48,95])                     (coord cases 1,0)
  C = 0.50*(x[95,47]+x[95,48])                     (coord cases 0,1)
  D =       x[95,95]                               (coord cases 1,1)

After the stride-3 VALID conv over the rearranged feature map, the output is
b_conv everywhere except the 2x2 corner (per batch, out-channel):

  out[b,o,0,0] = sum_c A*w00 + C*w01 + B*w10 + D*w11   (+ b_conv[o])
  out[b,o,0,1] = sum_c C*w00 + D*w10
  out[b,o,1,0] = sum_c B*w00 + D*w01
  out[b,o,1,1] = sum_c D*w00

(w_ij = w_conv[o,c,i,j]; the offset-conv branch is dead: + 0.0*sum(off).)

Sharding: output channels are split 8 ways across the NeuronCores (the batch
dim is only 4); the sampled rows of x (y=47,48,95) are replicated.  Each core
gathers its bilinear taps on-device (reduces + scales), runs the 4 corner
matmuls on the TensorEngine, fills its background tile and writes its
[4,8,96,96] output shard with disjoint DMAs spread over both HWDGE rings.

Two program variants: b_conv==0 (always true for this problem's
setup_inputs) uses a pure-memset background; nonzero b_conv broadcasts the
bias with a DVE copy (step-0 AP) instead.
"""

import numpy as np

B, IC, IH, IW = 4, 64, 96, 96
OC = 64
NCORES = 8
OCP = OC // NCORES  # out channels per core
HW = IH * IW        # 9216
QCH = HW // 4       # 2304: background tile free size (4 partition-chunks/plane)

_ROWS = (47, 48, 95)  # sampled rows of x (y coords); cols sampled: 47,48,95

_prog_cache = {}


def _build_program(mode):
    """One SPMD Bass program: identical on every core; per-core data differs.

    mode: "corners" writes only the 2x2 corner values and relies on
    run_bass_kernel_spmd's documented pre-zeroed ExternalOutput buffers
    (kernel() verifies that on the host and falls back to "zero_full");
    "zero_full"/"bias_full" write the whole shard.
    """
    import concourse.bacc as bacc
    import concourse.bass as bass
    import concourse.mybir as mybir
    import concourse.tile as tile

    with_bias = mode == "bias_full"
    corners_only = mode == "corners"

    nc = bacc.Bacc()
    dt = mybir.dt.float32

    xr_d = nc.declare_dram_parameter("xr", [IC, B, 3, 3], dt, isOutput=False)
    wb_d = nc.declare_dram_parameter("wb", [128, 64], dt, isOutput=False)
    if with_bias:
        b128_d = nc.declare_dram_parameter("bias128", [128, 1], dt, isOutput=False)
        bv_d = nc.declare_dram_parameter("biasV", [B, OCP], dt, isOutput=False)
    out_d = nc.declare_dram_parameter("out", [B, OCP, IH, IW], dt, isOutput=True)

    with tile.TileContext(nc) as tc:
        with (
            tc.tile_pool(name="sbuf", bufs=1) as pool,
            tc.tile_pool(name="psum", bufs=1, space=bass.MemorySpace.PSUM) as psum,
        ):
            xr = pool.tile([IC, B, 3, 3], dt)
            w2 = pool.tile([128, 64], dt)
            S3 = pool.tile([128, 4], dt)
            S4 = pool.tile([128, 4], dt)
            Vp = psum.tile([B, 32], dt)
            if corners_only:
                V = pool.tile([B, OCP, 2, 2], dt)
            else:
                bgh = pool.tile([B, OCP, 192], dt)
                bg = pool.tile([128, QCH], dt)

            # Background: bg[q*32 + b*8 + o, r] = b_conv[o] (or just zeros).
            # The fill gates the big output writes, so keep it off the Scalar
            # engine (ACT table load + drain costs ~6us) and split the plain
            # memset across DVE and GpSimd.
            # Input DMAs first on both rings: their ~2-3us HBM completion
            # fences gate the whole corner-value chain.
            nc.scalar.dma_start(xr[:], xr_d[:])
            nc.sync.dma_start(w2[:], wb_d[:])

            if with_bias:
                b128 = pool.tile([128, 1], dt)
                bo8 = pool.tile([B, OCP], dt)
                nc.scalar.dma_start(b128[:], b128_d[:])
                nc.scalar.dma_start(bo8[:], bv_d[:])
                nc.vector.tensor_copy(bg[:], b128[:, 0:1].to_broadcast((128, QCH)))
                nc.vector.tensor_copy(
                    bgh[:], bo8[:, :, None].to_broadcast((B, OCP, 192))
                )
            elif not corners_only:
                nc.vector.memset(bg[:, 0 : QCH // 2], 0.0)
                nc.gpsimd.memset(bg[:, QCH // 2 : QCH], 0.0)
                nc.gpsimd.memset(bgh[:], 0.0)

            # Bilinear tap sums (bilinear scale factors are folded into the
            # host-prepared weight matrix).  Taps stacked on the K axis:
            # S3 = [A (c rows 0:64) | B (rows 64:128)], S4 = [C | D],
            # free dim = b.  xr rows: 0->y47, 1->y48, 2->y95.
            AX = mybir.AxisListType
            nc.vector.reduce_sum(S3[0:64, :], xr[:, :, 0:2, 0:2], axis=AX.XY)
            nc.vector.reduce_sum(S3[64:128, :], xr[:, :, 0:2, 2:3], axis=AX.XY)
            nc.vector.reduce_sum(S4[0:64, :], xr[:, :, 2:3, 0:2], axis=AX.XY)
            nc.vector.tensor_copy(S4[64:128, :], xr[:, :, 2:3, 2:3])

            # Corner values Vp[b, (h*2+w)*8+o] in two K=128 matmuls; w2 holds
            # the scale-folded, zero-padded tap weights (cols 0:32 for A|B,
            # 32:64 for C|D).
            MM = nc.tensor.matmul
            MM(Vp[:], S3[:], w2[:, 0:32], start=True, stop=False)
            MM(Vp[:], S4[:], w2[:, 32:64], start=False, stop=True)

            if corners_only:
                # Only the corners are nonzero; the runner hands the NEFF
                # pre-zeroed output buffers, so write just those 1024 values
                # (idempotent under replay).
                nc.vector.tensor_copy(
                    V[:].rearrange("b o h w -> b o (h w)"),
                    Vp[:].rearrange("b (hw o) -> b o hw", o=OCP),
                )
                nc.sync.dma_start(out_d[:, :, 0:1, 0:2], V[:, :, 0:1, :])
                nc.scalar.dma_start(out_d[:, :, 1:2, 0:2], V[:, :, 1:2, :])
            else:
                # Add the corner values straight from PSUM into the q0-head
                # tile (rows h=0,1 of every (b,o) plane): bgh[b, o, h*96+w]
                # += Vp[b, (h*2+w)*8+o].  One DVE op; no small-run DMA ever
                # targets HBM (their completion fences are the kernel tail).
                bghv = bgh[:].rearrange("b o (h x w) -> b o h x w", h=2, x=48)[
                    :, :, :, 0, 0:2
                ]
                nc.vector.tensor_add(
                    bghv,
                    bghv,
                    Vp[:].rearrange("b (h w o) -> b o h w", h=2, w=2),
                )

                # Output shard writes, all disjoint: plane hw = q*2304 + r.
                # Alternate the two HWDGE rings (sync=SP, scalar=ACT).
                ovb = out_d[:].rearrange("b o h w -> b o (h w)")
                ov = out_d[:].rearrange("b o h w -> (b o) (h w)")
                nc.scalar.dma_start(ov[:, 192:QCH], bg[0:32, 192:QCH])
                nc.sync.dma_start(ov[:, QCH : 2 * QCH], bg[32:64, :])
                nc.scalar.dma_start(ov[:, 2 * QCH : 3 * QCH], bg[64:96, :])
                nc.sync.dma_start(ov[:, 3 * QCH : 4 * QCH], bg[96:128, :])
                # V-dependent head write last so it can't head-of-line block
                # the independent plane writes on its ring.
                nc.sync.dma_start(ovb[:, :, 0:192], bgh[:])

    nc.finalize()  # Bacc.finalize runs the wait-splitting legalization passes
    return nc


def _get_program(mode):
    if mode not in _prog_cache:
        _prog_cache[mode] = _build_program(mode)
    return _prog_cache[mode]


def _make_in_maps(x, w_conv, b_conv, with_bias=None):
    x = np.ascontiguousarray(x, dtype=np.float32)
    w_conv = np.ascontiguousarray(w_conv, dtype=np.float32)
    b_conv = np.ascontiguousarray(b_conv, dtype=np.float32)
    if with_bias is None:
        with_bias = bool(np.any(b_conv != 0))

    xr = np.ascontiguousarray(
        x[:, :, _ROWS, :][:, :, :, _ROWS].transpose(1, 0, 2, 3)
    )
    in_maps = []
    for core in range(NCORES):
        o0 = core * OCP
        wsl = w_conv[o0 : o0 + OCP, :, 0:2, 0:2]  # [8,64,2,2] (o,c,i,j)
        # w_pq[pq][c, o] helper: w_conv[o,c,i,j] transposed to [c,o]
        wco = lambda i, j: wsl[:, :, i, j].T  # [64(c), 8(o)]
        z = np.zeros((IC, OCP), np.float32)
        # Vp[b, pq*8+o] = sum_k S[k,b]*w2[k, pq*8+o]; K rows: A=0:64, B=64:128
        # (first matmul), C=0:64, D=64:128 (second).  Scales folded here.
        wA = np.concatenate([0.25 * wco(0, 0), z, z, z], axis=1)          # out00
        wB = np.concatenate([0.5 * wco(1, 0), z, 0.5 * wco(0, 0), z], axis=1)
        wC = np.concatenate([0.5 * wco(0, 1), 0.5 * wco(0, 0), z, z], axis=1)
        wD = np.concatenate([wco(1, 1), wco(1, 0), wco(0, 1), wco(0, 0)], axis=1)
        w2 = np.ascontiguousarray(
            np.concatenate(
                [np.concatenate([wA, wB], axis=0), np.concatenate([wC, wD], axis=0)],
                axis=1,
            )
        )  # [128, 64]
        m = {"xr": xr, "wb": w2}
        if with_bias:
            bc8 = b_conv[o0 : o0 + OCP]
            m["bias128"] = np.ascontiguousarray(np.tile(bc8, 16)[:, None])
            # biasV[b, o] = bc8[o]
            m["biasV"] = np.ascontiguousarray(
                np.broadcast_to(bc8[None, :], (B, OCP))
            )
        in_maps.append(m)
    return in_maps


def _run_mode(mode, x, w_conv, b_conv, trace=False, **spmd_kwargs):
    from concourse.bass_utils import run_bass_kernel_spmd

    nc = _get_program(mode)
    in_maps = _make_in_maps(x, w_conv, b_conv, mode == "bias_full")
    res = run_bass_kernel_spmd(
        nc, in_maps, core_ids=list(range(NCORES)), trace=trace, **spmd_kwargs
    )
    out = np.concatenate([r["out"] for r in res.results], axis=1)
    return out, res


def _background_is_zero(out):
    # Everything except the 2x2 corner of each (b, o) plane must be zero.
    return (
        np.count_nonzero(out[:, :, 2:, :]) == 0
        and np.count_nonzero(out[:, :, :2, 2:]) == 0
    )


def _run(x, w_conv, b_conv, trace=False, **spmd_kwargs):
    if bool(np.any(np.asarray(b_conv) != 0)):
        return _run_mode("bias_full", x, w_conv, b_conv, trace, **spmd_kwargs)
    out, res = _run_mode("corners", x, w_conv, b_conv, trace, **spmd_kwargs)
    if _background_is_zero(out):
        return out, res
    # Pre-zeroed-output contract did not hold; redo with full writes.
    return _run_mode("zero_full", x, w_conv, b_conv, trace, **spmd_kwargs)


def kernel(x, w_off, b_off, w_conv, b_conv):
    out, _ = _run(x, w_conv, b_conv, trace=False)
    return out


# revision 19
# speedup vs baseline: 1.2777x; 1.0308x over previous
"""Trainium2 Bass kernel for nn_DeformConv_23278722744918.

The reference passes raw integer pixel coordinates to grid_sample as if they
were normalized [-1,1] coords (align_corners=True). After de-normalization,
xpix = (clip(h+i,0,95)+1)*47.5 and ypix = (clip(w+j,0,95)+1)*47.5, so every
sample with h+i >= 2 or w+j >= 2 lands outside [0,95] and is zero
(padding_mode='zeros').  Only four tap values survive, shared by all (h,w):

  A = 0.25*(x[47,47]+x[47,48]+x[48,47]+x[48,48])   (coord cases 0,0)
  B = 0.50*(x[47,95]+x[48,95])                     (coord cases 1,0)
  C = 0.50*(x[95,47]+x[95,48])                     (coord cases 0,1)
  D =       x[95,95]                               (coord cases 1,1)

After the stride-3 VALID conv over the rearranged feature map, the output is
b_conv everywhere except the 2x2 corner (per batch, out-channel):

  out[b,o,0,0] = sum_c A*w00 + C*w01 + B*w10 + D*w11   (+ b_conv[o])
  out[b,o,0,1] = sum_c C*w00 + D*w10
  out[b,o,1,0] = sum_c B*w00 + D*w01
  out[b,o,1,1] = sum_c D*w00

(w_ij = w_conv[o,c,i,j]; the offset-conv branch is dead: + 0.0*sum(off).)

Sharding: output channels are split 8 ways across the NeuronCores (the batch
dim is only 4); the sampled rows of x (y=47,48,95) are replicated.  Each core
gathers its bilinear taps on-device (reduces + scales), runs the 4 corner
matmuls on the TensorEngine, fills its background tile and writes its
[4,8,96,96] output shard with disjoint DMAs spread over both HWDGE rings.

Two program variants: b_conv==0 (always true for this problem's
setup_inputs) uses a pure-memset background; nonzero b_conv broadcasts the
bias with a DVE copy (step-0 AP) instead.
"""

import numpy as np

B, IC, IH, IW = 4, 64, 96, 96
OC = 64
NCORES = 8
OCP = OC // NCORES  # out channels per core
HW = IH * IW        # 9216
QCH = HW // 4       # 2304: background tile free size (4 partition-chunks/plane)

_ROWS = (47, 48, 95)  # sampled rows of x (y coords); cols sampled: 47,48,95

_prog_cache = {}


def _build_program(mode):
    """One SPMD Bass program: identical on every core; per-core data differs.

    mode: "corners" writes only the 2x2 corner values and relies on
    run_bass_kernel_spmd's documented pre-zeroed ExternalOutput buffers
    (kernel() verifies that on the host and falls back to "zero_full");
    "zero_full"/"bias_full" write the whole shard.
    """
    import concourse.bacc as bacc
    import concourse.bass as bass
    import concourse.mybir as mybir
    import concourse.tile as tile

    with_bias = mode == "bias_full"
    corners_only = mode == "corners"

    nc = bacc.Bacc()
    dt = mybir.dt.float32

    if corners_only:
        xr_d = nc.declare_dram_parameter("xr", [128, 2 * B, 4], dt, isOutput=False)
    else:
        xr_d = nc.declare_dram_parameter("xr", [IC, B, 3, 3], dt, isOutput=False)
    wb_d = nc.declare_dram_parameter("wb", [128, 64], dt, isOutput=False)
    if with_bias:
        b128_d = nc.declare_dram_parameter("bias128", [128, 1], dt, isOutput=False)
        bv_d = nc.declare_dram_parameter("biasV", [B, OCP], dt, isOutput=False)
    out_d = nc.declare_dram_parameter("out", [B, OCP, IH, IW], dt, isOutput=True)

    with tile.TileContext(nc) as tc:
        with (
            tc.tile_pool(name="sbuf", bufs=1) as pool,
            tc.tile_pool(name="psum", bufs=1, space=bass.MemorySpace.PSUM) as psum,
        ):
            w2 = pool.tile([128, 64], dt)
            Vp = psum.tile([B, 32], dt)
            if corners_only:
                xr = pool.tile([128, 2 * B, 4], dt)
                S34 = pool.tile([128, 2 * B], dt)
                V = pool.tile([B, OCP, 2, 2], dt)
            else:
                xr = pool.tile([IC, B, 3, 3], dt)
                S3 = pool.tile([128, 4], dt)
                S4 = pool.tile([128, 4], dt)
                bgh = pool.tile([B, OCP, 192], dt)
                bg = pool.tile([128, QCH], dt)

            # Background: bg[q*32 + b*8 + o, r] = b_conv[o] (or just zeros).
            # The fill gates the big output writes, so keep it off the Scalar
            # engine (ACT table load + drain costs ~6us) and split the plain
            # memset across DVE and GpSimd.
            # Input DMAs first on both rings: their ~2-3us HBM completion
            # fences gate the whole corner-value chain.
            nc.scalar.dma_start(xr[:], xr_d[:])
            nc.sync.dma_start(w2[:], wb_d[:])

            if with_bias:
                b128 = pool.tile([128, 1], dt)
                bo8 = pool.tile([B, OCP], dt)
                nc.scalar.dma_start(b128[:], b128_d[:])
                nc.scalar.dma_start(bo8[:], bv_d[:])
                nc.vector.tensor_copy(bg[:], b128[:, 0:1].to_broadcast((128, QCH)))
                nc.vector.tensor_copy(
                    bgh[:], bo8[:, :, None].to_broadcast((B, OCP, 192))
                )
            elif not corners_only:
                nc.vector.memset(bg[:, 0 : QCH // 2], 0.0)
                nc.gpsimd.memset(bg[:, QCH // 2 : QCH], 0.0)
                nc.gpsimd.memset(bgh[:], 0.0)

            # Bilinear tap sums (bilinear scale factors are folded into the
            # host-prepared weight matrix).  Taps stacked on the K axis:
            # S3 = [A (c rows 0:64) | B (rows 64:128)], S4 = [C | D],
            # free dim = b.  xr rows: 0->y47, 1->y48, 2->y95.
            AX = mybir.AxisListType
            if corners_only:
                # One reduce: S34[k, s*4+b] = sum_j xr[k, s*4+b, j]; the host
                # packs the bilinear tap pixels (zero-padded) so rows 0:64
                # are per-channel A (s=0) / C (s=1) and 64:128 are B / D.
                nc.vector.reduce_sum(S34[:], xr[:], axis=AX.X)
                Sa, Sb = S34[:, 0:4], S34[:, 4:8]
            else:
                nc.vector.reduce_sum(S3[0:64, :], xr[:, :, 0:2, 0:2], axis=AX.XY)
                nc.vector.reduce_sum(S3[64:128, :], xr[:, :, 0:2, 2:3], axis=AX.XY)
                nc.vector.reduce_sum(S4[0:64, :], xr[:, :, 2:3, 0:2], axis=AX.XY)
                nc.vector.tensor_copy(S4[64:128, :], xr[:, :, 2:3, 2:3])
                Sa, Sb = S3[:], S4[:]

            # Corner values Vp[b, (h*2+w)*8+o] in two K=128 matmuls; w2 holds
            # the scale-folded, zero-padded tap weights (cols 0:32 for A|B,
            # 32:64 for C|D).
            MM = nc.tensor.matmul
            MM(Vp[:], Sa, w2[:, 0:32], start=True, stop=False)
            MM(Vp[:], Sb, w2[:, 32:64], start=False, stop=True)

            if corners_only:
                # Only the corners are nonzero; the runner hands the NEFF
                # pre-zeroed output buffers, so write just those 1024 values
                # (idempotent under replay).
                nc.vector.tensor_copy(
                    V[:].rearrange("b o h w -> b o (h w)"),
                    Vp[:].rearrange("b (hw o) -> b o hw", o=OCP),
                )
                nc.sync.dma_start(out_d[:, :, 0:1, 0:2], V[:, :, 0:1, :])
                nc.scalar.dma_start(out_d[:, :, 1:2, 0:2], V[:, :, 1:2, :])
            else:
                # Add the corner values straight from PSUM into the q0-head
                # tile (rows h=0,1 of every (b,o) plane): bgh[b, o, h*96+w]
                # += Vp[b, (h*2+w)*8+o].  One DVE op; no small-run DMA ever
                # targets HBM (their completion fences are the kernel tail).
                bghv = bgh[:].rearrange("b o (h x w) -> b o h x w", h=2, x=48)[
                    :, :, :, 0, 0:2
                ]
                nc.vector.tensor_add(
                    bghv,
                    bghv,
                    Vp[:].rearrange("b (h w o) -> b o h w", h=2, w=2),
                )

                # Output shard writes, all disjoint: plane hw = q*2304 + r.
                # Alternate the two HWDGE rings (sync=SP, scalar=ACT).
                ovb = out_d[:].rearrange("b o h w -> b o (h w)")
                ov = out_d[:].rearrange("b o h w -> (b o) (h w)")
                nc.scalar.dma_start(ov[:, 192:QCH], bg[0:32, 192:QCH])
                nc.sync.dma_start(ov[:, QCH : 2 * QCH], bg[32:64, :])
                nc.scalar.dma_start(ov[:, 2 * QCH : 3 * QCH], bg[64:96, :])
                nc.sync.dma_start(ov[:, 3 * QCH : 4 * QCH], bg[96:128, :])
                # V-dependent head write last so it can't head-of-line block
                # the independent plane writes on its ring.
                nc.sync.dma_start(ovb[:, :, 0:192], bgh[:])

    nc.finalize()  # Bacc.finalize runs the wait-splitting legalization passes
    return nc


def _get_program(mode):
    if mode not in _prog_cache:
        _prog_cache[mode] = _build_program(mode)
    return _prog_cache[mode]


def _make_in_maps(x, w_conv, b_conv, with_bias=None, corners_only=False):
    x = np.ascontiguousarray(x, dtype=np.float32)
    w_conv = np.ascontiguousarray(w_conv, dtype=np.float32)
    b_conv = np.ascontiguousarray(b_conv, dtype=np.float32)
    if with_bias is None:
        with_bias = bool(np.any(b_conv != 0))

    xs = x[:, :, _ROWS, :][:, :, :, _ROWS].transpose(1, 0, 2, 3)  # [c,b,3,3]
    if corners_only:
        # Pack tap pixels for the single on-device reduce (see builder).
        xr = np.zeros((128, 2 * B, 4), np.float32)
        xr[0:64, 0:4, 0:4] = xs[:, :, 0:2, 0:2].reshape(IC, B, 4)  # A
        xr[64:128, 0:4, 0:2] = xs[:, :, 0:2, 2]                    # B
        xr[0:64, 4:8, 0:2] = xs[:, :, 2, 0:2]                      # C
        xr[64:128, 4:8, 0:1] = xs[:, :, 2, 2:3]                    # D
    else:
        xr = np.ascontiguousarray(xs)
    in_maps = []
    for core in range(NCORES):
        o0 = core * OCP
        wsl = w_conv[o0 : o0 + OCP, :, 0:2, 0:2]  # [8,64,2,2] (o,c,i,j)
        # w_pq[pq][c, o] helper: w_conv[o,c,i,j] transposed to [c,o]
        wco = lambda i, j: wsl[:, :, i, j].T  # [64(c), 8(o)]
        z = np.zeros((IC, OCP), np.float32)
        # Vp[b, pq*8+o] = sum_k S[k,b]*w2[k, pq*8+o]; K rows: A=0:64, B=64:128
        # (first matmul), C=0:64, D=64:128 (second).  Scales folded here.
        wA = np.concatenate([0.25 * wco(0, 0), z, z, z], axis=1)          # out00
        wB = np.concatenate([0.5 * wco(1, 0), z, 0.5 * wco(0, 0), z], axis=1)
        wC = np.concatenate([0.5 * wco(0, 1), 0.5 * wco(0, 0), z, z], axis=1)
        wD = np.concatenate([wco(1, 1), wco(1, 0), wco(0, 1), wco(0, 0)], axis=1)
        w2 = np.ascontiguousarray(
            np.concatenate(
                [np.concatenate([wA, wB], axis=0), np.concatenate([wC, wD], axis=0)],
                axis=1,
            )
        )  # [128, 64]
        m = {"xr": xr, "wb": w2}
        if with_bias:
            bc8 = b_conv[o0 : o0 + OCP]
            m["bias128"] = np.ascontiguousarray(np.tile(bc8, 16)[:, None])
            # biasV[b, o] = bc8[o]
            m["biasV"] = np.ascontiguousarray(
                np.broadcast_to(bc8[None, :], (B, OCP))
            )
        in_maps.append(m)
    return in_maps


def _run_mode(mode, x, w_conv, b_conv, trace=False, **spmd_kwargs):
    from concourse.bass_utils import run_bass_kernel_spmd

    nc = _get_program(mode)
    in_maps = _make_in_maps(
        x, w_conv, b_conv, mode == "bias_full", corners_only=mode == "corners"
    )
    res = run_bass_kernel_spmd(
        nc, in_maps, core_ids=list(range(NCORES)), trace=trace, **spmd_kwargs
    )
    out = np.concatenate([r["out"] for r in res.results], axis=1)
    return out, res


def _background_is_zero(out):
    # Everything except the 2x2 corner of each (b, o) plane must be zero.
    return (
        np.count_nonzero(out[:, :, 2:, :]) == 0
        and np.count_nonzero(out[:, :, :2, 2:]) == 0
    )


def _run(x, w_conv, b_conv, trace=False, **spmd_kwargs):
    if bool(np.any(np.asarray(b_conv) != 0)):
        return _run_mode("bias_full", x, w_conv, b_conv, trace, **spmd_kwargs)
    out, res = _run_mode("corners", x, w_conv, b_conv, trace, **spmd_kwargs)
    if _background_is_zero(out):
        return out, res
    # Pre-zeroed-output contract did not hold; redo with full writes.
    return _run_mode("zero_full", x, w_conv, b_conv, trace, **spmd_kwargs)


def kernel(x, w_off, b_off, w_conv, b_conv):
    out, _ = _run(x, w_conv, b_conv, trace=False)
    return out
